# revision 1
# baseline (speedup 1.0000x reference)
"""Trainium2 Bass kernel for windowed attention with LoRA + decomposed rel-pos bias.

Full-input contract: kernel(**inputs) takes the unsharded numpy inputs and
returns the full (64, 14, 14, 768) float32 output.

Strategy (8 NeuronCores, data-parallel over the 64-window batch, 8 windows/core):
  Host prep (numpy):
    - Fold LoRA into qkv weights:  Wq += lb_q@la_q, Wv += lb_v@la_v  (exact math).
    - Fold attention scale (2^-3, exact) into Wq / b_q; rel-pos tables get 1/scale.
    - Pre-transpose all weights + x so every on-chip matmul operand has its
      contraction dim on SBUF partitions (no on-chip transposes at all).
    - Gather rel_pos tables with the (q-k) index map; cast everything to bf16.
  On chip (per core, all SBUF resident):
    - qk projection -> per-(window, head) "augmented" q/k tiles [128, 196]:
      rows hold q (or k) in one 64-row half plus 14 rel-pos feature rows and
      14 one-hot rows so that ONE matmul per key-chunk produces
      q@k^T*scale + rel_h + rel_w directly in PSUM (K-augmentation trick).
    - exp on ScalarE (softmax without max-subtraction: logits are O(1)).
    - attn@v with an appended ones-column on v so the softmax denominator
      falls out of the same matmul; normalize with a reciprocal + DRAM-bounce
      DMA partition-broadcast + one VectorE multiply.
    - head-major attention interleaved with the rel-feature stage so VectorE
      copy work overlaps TensorE matmuls; projection reads a persistent
      all-window out2 tile allocated in the space freed by the xT pool.
"""

import numpy as np
import ml_dtypes

B_TOTAL = 64
NCORES = 8
BPC = B_TOTAL // NCORES  # windows per core
H = W = 14
N = H * W  # 196 tokens per window
DIM = 768
NH = 12
HD = 64
DC = DIM // 128  # 6 contraction chunks
NKT0, NKT1 = 128, N - 128  # key-token chunks (128 + 68)
SCALE = HD ** -0.5  # 0.125, exact power of two

# row maps inside the 128-partition augmented q/k tiles
# even head parity: q/k rows 0:64, relh/kh-onehot 64:78, zeros 78:96,
#                   relw/kw-onehot 96:110; contraction range [0:110)
# odd  head parity: relw/kw-onehot 0:14, zeros 14:32, relh/kh-onehot 32:46,
#                   zeros 46:64, q/k rows 64:128; contraction range [0:128)
K_EVEN = 110
K_ODD = 128

_NC_CACHE = {}


def build_module(debug=False):
    from contextlib import ExitStack

    import concourse.tile as tile
    from concourse import bacc, mybir

    f32 = mybir.dt.float32
    bf16 = mybir.dt.bfloat16
    AF = mybir.ActivationFunctionType
    ALU = mybir.AluOpType

    nc = bacc.Bacc(
        "TRN2", target_bir_lowering=False, debug=False, num_devices=NCORES
    )

    T = BPC * N  # 1568 tokens per core

    xT = nc.dram_tensor("xT", [DIM, T], bf16, kind="ExternalInput").ap()
    wqk = nc.dram_tensor("wqk", [DIM, 2 * DIM], bf16, kind="ExternalInput").ap()
    wv = nc.dram_tensor("wv", [DIM, DIM], bf16, kind="ExternalInput").ap()
    pw = nc.dram_tensor("pw", [DIM, DIM], bf16, kind="ExternalInput").ap()
    bqk = nc.dram_tensor("bqk", [2 * DIM], f32, kind="ExternalInput").ap()
    bv = nc.dram_tensor("bv", [DIM], bf16, kind="ExternalInput").ap()
    bp = nc.dram_tensor("bp", [DIM], bf16, kind="ExternalInput").ap()
    relh = nc.dram_tensor("relh", [HD, N], bf16, kind="ExternalInput").ap()
    relw = nc.dram_tensor("relw", [HD, N], bf16, kind="ExternalInput").ap()
    oh_e = nc.dram_tensor("oh_e", [46, N], bf16, kind="ExternalInput").ap()
    oh_o = nc.dram_tensor("oh_o", [64, N], bf16, kind="ExternalInput").ap()
    zer = nc.dram_tensor("zer", [18, N], bf16, kind="ExternalInput").ap()
    out = nc.dram_tensor("out", [T, DIM], f32, kind="ExternalOutput").ap()
    if debug:
        d_qaug = nc.dram_tensor(
            "d_qaug", [128, BPC * NH, N], bf16, kind="ExternalOutput"
        ).ap()
        d_kaug = nc.dram_tensor(
            "d_kaug", [128, BPC * NH, N], bf16, kind="ExternalOutput"
        ).ap()
        d_vall = nc.dram_tensor(
            "d_vall", [128, BPC * 2 * NH * (HD + 1)], bf16, kind="ExternalOutput"
        ).ap()
        d_o2 = nc.dram_tensor(
            "d_o2", [128, BPC, DC, N], bf16, kind="ExternalOutput"
        ).ap()

    with tile.TileContext(nc) as tc, ExitStack() as ctx:
        singles = ctx.enter_context(tc.tile_pool(name="singles", bufs=1))
        ps = ctx.enter_context(tc.tile_pool(name="ps", bufs=5, space="PSUM"))
        psd = ctx.enter_context(tc.tile_pool(name="psd", bufs=2, space="PSUM"))
        pdp = ctx.enter_context(tc.tile_pool(name="pdp", bufs=1, space="PSUM"))
        attn_pool = ctx.enter_context(tc.tile_pool(name="attn", bufs=3))
        r_pool = ctx.enter_context(tc.tile_pool(name="rp", bufs=2))
        osb_pool = ctx.enter_context(tc.tile_pool(name="osb", bufs=2))
        rd_pool = ctx.enter_context(tc.tile_pool(name="rd", bufs=2, space="DRAM"))
        xt_pool_cm = tc.tile_pool(name="xt", bufs=1)
        xt_pool = xt_pool_cm.__enter__()

        # ---- resident SBUF tensors ----
        wqk_sb = singles.tile([128, DC, 2 * DIM], bf16)
        wqk_r = wqk.rearrange("(c p) o -> p c o", p=128)
        for c in range(DC):
            nc.sync.dma_start(out=wqk_sb[:, c, :], in_=wqk_r[:, c, :])
        wv_sb = singles.tile([128, DC, DIM], bf16)
        nc.sync.dma_start(out=wv_sb[:], in_=wv.rearrange("(c p) o -> p c o", p=128))
        pw_sb = singles.tile([128, DC, DIM], bf16)
        nc.sync.dma_start(out=pw_sb[:], in_=pw.rearrange("(c p) o -> p c o", p=128))
        bqk_sb = singles.tile([128, 2 * DC], f32)
        nc.sync.dma_start(out=bqk_sb[:], in_=bqk.rearrange("(c p) -> p c", p=128))
        bv_sb = singles.tile([128, DIM], bf16)
        nc.sync.dma_start(out=bv_sb[:], in_=bv.unsqueeze(0).broadcast_to([128, DIM]))
        bp_sb = singles.tile([128, DIM], bf16)
        nc.sync.dma_start(out=bp_sb[:], in_=bp.unsqueeze(0).broadcast_to([128, DIM]))
        relh_sb = singles.tile([128, N], bf16)
        nc.sync.dma_start(out=relh_sb[0:64, :], in_=relh)
        nc.sync.dma_start(out=relh_sb[64:128, :], in_=relh)
        relw_sb = singles.tile([128, N], bf16)
        nc.sync.dma_start(out=relw_sb[0:64, :], in_=relw)
        nc.sync.dma_start(out=relw_sb[64:128, :], in_=relw)

        NPAIR = BPC * NH  # 96
        qaug = singles.tile([128, NPAIR, N], bf16)
        kaug = singles.tile([128, NPAIR, N], bf16)
        # [t-chunk partitions, window, chunk, head, hd+ones]
        vall = singles.tile([128, BPC, 2, NH, HD + 1], bf16)
        nc.vector.memset(vall[:, :, :, :, HD : HD + 1], 1.0)
        if debug:
            nc.vector.memset(qaug[:], 0.0)
            nc.vector.memset(kaug[:], 0.0)
            nc.vector.memset(vall[:], 0.0)
            nc.vector.memset(vall[:, :, :, :, HD : HD + 1], 1.0)

        # xT lives only through the projection phases; its pool is released
        # afterwards so the persistent out2 tile can reuse the space.
        xT_sb = xt_pool.tile([128, DC, T], bf16)
        xT_r = xT.rearrange("(c p) t -> p c t", p=128)
        for c in range(DC):
            nc.sync.dma_start(out=xT_sb[:, c, :], in_=xT_r[:, c, :])

        # views
        qv = qaug.rearrange(
            "p (b hh par) q -> p b hh par q", b=BPC, hh=NH // 2, par=2
        )
        qv6 = qaug.rearrange(
            "p (b hh par) (qh qw) -> p b hh par qh qw",
            b=BPC, hh=NH // 2, par=2, qh=H,
        )
        NPR = BPC * NH // 2  # 48 even/odd pair slots
        qpv = qaug.rearrange("p (pr par) q -> p pr par q", par=2)
        kpv = kaug.rearrange("p (pr par) q -> p pr par q", par=2)

        def bcast(src_ap, rows):
            return src_ap[0:rows].rearrange("j q -> j () q").broadcast_to(
                [rows, NPR, N]
            )

        nc.sync.dma_start(out=kpv[64:110, :, 0, :], in_=bcast(oh_e, 46))
        nc.sync.dma_start(out=kpv[0:64, :, 1, :], in_=bcast(oh_o, 64))
        nc.sync.dma_start(out=qpv[78:96, :, 0, :], in_=bcast(zer, 18))
        nc.sync.dma_start(out=qpv[14:32, :, 1, :], in_=bcast(zer, 18))
        nc.sync.dma_start(out=qpv[46:64, :, 1, :], in_=bcast(zer, 18))

        qp = qaug
        kp = kaug

        # ---- phase 1: q/k projection (two windows per psum tile) ----
        dest_v = [
            qaug.rearrange("p (b2 w2 h) q -> p b2 w2 h q", w2=2, h=NH),
            kaug.rearrange("p (b2 w2 h) q -> p b2 w2 h q", w2=2, h=NH),
        ]
        for b2 in range(BPC // 2):
            for oc in range(2 * DC):  # 6 q chunks then 6 k chunks
                p_qk = ps.tile([128, 512], f32, tag="ps")
                for dc in range(DC):
                    nc.tensor.matmul(
                        p_qk[:, 0 : 2 * N],
                        lhsT=wqk_sb[:, dc, oc * 128 : (oc + 1) * 128],
                        rhs=xT_sb[:, dc, 2 * b2 * N : (2 * b2 + 2) * N],
                        start=(dc == 0),
                        stop=(dc == DC - 1),
                    )
                is_q = oc < DC
                hh = (oc % DC) * 2
                dv = dest_v[0] if is_q else dest_v[1]
                for par in range(2):
                    h = hh + par
                    rows = slice(0, 64) if par == 0 else slice(64, 128)
                    nc.scalar.activation(
                        out=dv[rows, b2, :, h, :],
                        in_=p_qk[rows, 0 : 2 * N].rearrange("p (w q) -> p w q", w=2),
                        func=AF.Identity,
                        bias=bqk_sb[rows, oc : oc + 1],
                        scale=1.0,
                    )

        # ---- phase 1b: v projection (natural layout, per window) ----
        for b in range(BPC):
            for i in range(2):  # token chunk within window: 128 / 68
                tc_rows = NKT0 if i == 0 else NKT1
                t0 = b * N + i * 128
                for half in range(2):
                    p_v = ps.tile([128, 512], f32, tag="ps")
                    for dc in range(DC):
                        nc.tensor.matmul(
                            p_v[0:tc_rows, 0:384],
                            lhsT=xT_sb[:, dc, t0 : t0 + tc_rows],
                            rhs=wv_sb[:, dc, half * 384 : (half + 1) * 384],
                            start=(dc == 0),
                            stop=(dc == DC - 1),
                        )
                    nc.vector.tensor_tensor(
                        out=vall[0:tc_rows, b, i, 6 * half : 6 * half + 6, 0:HD],
                        in0=p_v[0:tc_rows, 0:384].rearrange("p (h d) -> p h d", h=6),
                        in1=bv_sb[0:tc_rows, half * 384 : (half + 1) * 384].rearrange(
                            "p (h d) -> p h d", h=6
                        ),
                        op=ALU.add,
                    )

        # xT no longer needed; free its zone for o2_all
        xt_pool_cm.__exit__(None, None, None)
        o2_pool = ctx.enter_context(tc.tile_pool(name="o2", bufs=1))
        o2_all = o2_pool.tile([128, DC, T], bf16)

        # ---- phases 2+3, head-PAIR major: rel features then attention
        #      for both parities of a chunk, sharing one AV psum tile ----
        def emit_rel(hx):
            par = hx % 2
            q_rows = slice(0, 64) if par == 0 else slice(64, 128)
            lh_base = 0 if par == 0 else 64
            relh_rows = slice(64, 78) if par == 0 else slice(32, 46)
            relw_rows = slice(96, 110) if par == 0 else slice(0, 14)
            relh_tp = (lh_base, 64 if par == 0 else 32)
            relw_tp = (lh_base, 96 if par == 0 else 0)
            hh, hp = hx // 2, hx % 2
            for g2 in range(H // 2):
                g0 = 2 * g2
                p_r = psd.tile([128, 4, 128], f32, tag="psd")
                for s in range(2):
                    g = g0 + s
                    nc.tensor.matmul(
                        p_r[relh_rows, s, 0 : BPC * W],
                        lhsT=relh_sb[q_rows, g * W : (g + 1) * W],
                        rhs=qv[q_rows, :, hh, hp, g * W : (g + 1) * W],
                        start=True,
                        stop=True,
                        tile_position=relh_tp,
                    )
                    nc.tensor.matmul(
                        p_r[relw_rows, 2 + s, 0 : BPC * W],
                        lhsT=relw_sb[q_rows, g * W : (g + 1) * W],
                        rhs=qv[q_rows, :, hh, hp, g : g + 13 * W + 1 : W],
                        start=True,
                        stop=True,
                        tile_position=relw_tp,
                    )
                nc.vector.tensor_copy(
                    out=qv6[relh_rows, :, hh, hp, g0 : g0 + 2, :],
                    in_=p_r[relh_rows, 0:2, 0 : BPC * W].rearrange(
                        "p s (b w) -> p b s w", b=BPC
                    ),
                )
                nc.vector.tensor_copy(
                    out=qv6[relw_rows, :, hh, hp, :, g0 : g0 + 2],
                    in_=p_r[relw_rows, 2:4, 0 : BPC * W].rearrange(
                        "p s (b q) -> p b q s", b=BPC
                    ),
                )

        def emit_qk_exp(b, hx, a_sb):
            par = hx % 2
            pair = b * NH + hx
            krange = slice(0, K_EVEN) if par == 0 else slice(0, K_ODD)
            p_a = ps.tile([128, 2, 256], f32, tag="ps")
            nc.tensor.matmul(
                p_a[:, 0, 0:N],
                lhsT=kp[krange, pair, 0:NKT0],
                rhs=qp[krange, pair, :],
                start=True,
                stop=True,
            )
            nc.tensor.matmul(
                p_a[0:NKT1, 1, 0:N],
                lhsT=kp[krange, pair, NKT0:N],
                rhs=qp[krange, pair, :],
                start=True,
                stop=True,
            )
            nc.scalar.activation(
                out=a_sb[:, 0, :], in_=p_a[:, 0, 0:N], func=AF.Exp, scale=1.0
            )
            nc.scalar.activation(
                out=a_sb[0:NKT1, 1, :],
                in_=p_a[0:NKT1, 1, 0:N],
                func=AF.Exp,
                scale=1.0,
            )

        def emit_denom(b, hx, a_sb, pdf, r_hh):
            par = hx % 2
            d_row = 64 if par == 0 else 0
            d_tp = (0, 64) if par == 0 else (0, 0)
            nc.tensor.matmul(
                pdf[d_row : d_row + 1, 0:N],
                lhsT=vall[0:NKT0, b, 0, hx, HD : HD + 1],
                rhs=a_sb[:, 0, :],
                start=True,
                stop=False,
                tile_position=d_tp,
            )
            nc.tensor.matmul(
                pdf[d_row : d_row + 1, 0:N],
                lhsT=vall[0:NKT1, b, 1, hx, HD : HD + 1],
                rhs=a_sb[0:NKT1, 1, :],
                start=False,
                stop=True,
                tile_position=d_tp,
            )
            with nc.allow_low_precision(reason="bf16 softmax recip"):
                nc.vector.reciprocal(
                    out=r_hh[d_row : d_row + 1, b, :],
                    in_=pdf[d_row : d_row + 1, 0:N],
                )

        def emit_av(b, hx, a_sb, p_o):
            par = hx % 2
            rows = slice(0, 64) if par == 0 else slice(64, 128)
            av_tp = (0, 0) if par == 0 else (0, 64)
            nc.tensor.matmul(
                p_o[rows, 0:N],
                lhsT=vall[0:NKT0, b, 0, hx, 0:HD],
                rhs=a_sb[:, 0, :],
                start=True,
                stop=False,
                tile_position=av_tp,
                skip_group_check=True,
            )
            nc.tensor.matmul(
                p_o[rows, 0:N],
                lhsT=vall[0:NKT1, b, 1, hx, 0:HD],
                rhs=a_sb[0:NKT1, 1, :],
                start=False,
                stop=True,
                tile_position=av_tp,
                skip_group_check=True,
            )

        for hh in range(NH // 2):
            h0, h1 = 2 * hh, 2 * hh + 1
            emit_rel(h0)
            emit_rel(h1)
            r_hh = r_pool.tile([65, BPC, N], bf16, tag="rw")
            for b in range(BPC):
                a_sb0 = attn_pool.tile([128, 2, N], bf16, tag="a0")
                a_sb1 = attn_pool.tile([128, 2, N], bf16, tag="a1")
                emit_qk_exp(b, h0, a_sb0)
                emit_qk_exp(b, h1, a_sb1)
                p_dd = pdp.tile([128, 4, 128], f32, tag="pdp")
                pdf = p_dd.rearrange("p s c -> p (s c)")
                emit_denom(b, h0, a_sb0, pdf, r_hh)
                emit_denom(b, h1, a_sb1, pdf, r_hh)
                p_o = ps.tile([128, 512], f32, tag="ps")
                emit_av(b, h0, a_sb0, p_o)
                emit_av(b, h1, a_sb1, p_o)
                nc.scalar.activation(
                    out=o2_all[:, hh, b * N : (b + 1) * N],
                    in_=p_o[:, 0:N],
                    func=AF.Copy,
                    scale=1.0,
                )

            # broadcast reciprocals for both parities (DRAM bounce)
            dd = rd_pool.tile([2, BPC, N], bf16, tag="rd")
            rb_hh = r_pool.tile([128, BPC, N], bf16, tag="rb")
            nc.sync.dma_start(out=dd[0:1, :, :], in_=r_hh[64:65, :, :])
            nc.sync.dma_start(out=dd[1:2, :, :], in_=r_hh[0:1, :, :])
            nc.sync.dma_start(
                out=rb_hh[0:64, :, :], in_=dd[0:1, :, :].broadcast_to([64, BPC, N])
            )
            nc.sync.dma_start(
                out=rb_hh[64:128, :, :],
                in_=dd[1:2, :, :].broadcast_to([64, BPC, N]),
            )
            nc.vector.tensor_tensor(
                out=o2_all[:, hh, :].rearrange("p (b q) -> p b q", b=BPC),
                in0=o2_all[:, hh, :].rearrange("p (b q) -> p b q", b=BPC),
                in1=rb_hh[:, :, :],
                op=ALU.mult,
            )

        # ---- phase 4: projection over global 128-token chunks ----
        NT_CH = (T + 127) // 128  # 13
        for j in range(NT_CH):
            t0 = j * 128
            tc_rows = min(128, T - t0)
            o_sb = osb_pool.tile([128, DIM], f32, tag="osb")
            for half in range(2):
                p_p = ps.tile([128, 512], f32, tag="ps")
                for cc in range(DC):
                    nc.tensor.matmul(
                        p_p[0:tc_rows, 0:384],
                        lhsT=o2_all[:, cc, t0 : t0 + tc_rows],
                        rhs=pw_sb[:, cc, half * 384 : (half + 1) * 384],
                        start=(cc == 0),
                        stop=(cc == DC - 1),
                    )
                nc.vector.tensor_tensor(
                    out=o_sb[0:tc_rows, half * 384 : (half + 1) * 384],
                    in0=p_p[0:tc_rows, 0:384],
                    in1=bp_sb[0:tc_rows, half * 384 : (half + 1) * 384],
                    op=ALU.add,
                )
            nc.sync.dma_start(
                out=out[t0 : t0 + tc_rows, :],
                in_=o_sb[0:tc_rows, :],
            )

        if debug:
            nc.sync.dma_start(out=d_qaug, in_=qaug[:])
            nc.sync.dma_start(out=d_kaug, in_=kaug[:])
            nc.sync.dma_start(
                out=d_vall, in_=vall.rearrange("p a b c d -> p (a b c d)")
            )

    nc.finalize()
    return nc


def _host_prep(inputs):
    bf16 = ml_dtypes.bfloat16
    x = np.asarray(inputs["x"], np.float32)
    qkv_w = np.asarray(inputs["qkv_w"], np.float32)
    qkv_b = np.asarray(inputs["qkv_b"], np.float32)
    proj_w = np.asarray(inputs["proj_w"], np.float32)
    proj_b = np.asarray(inputs["proj_b"], np.float32)
    la_q = np.asarray(inputs["la_q"], np.float32)
    lb_q = np.asarray(inputs["lb_q"], np.float32)
    la_v = np.asarray(inputs["la_v"], np.float32)
    lb_v = np.asarray(inputs["lb_v"], np.float32)
    rel_pos_h = np.asarray(inputs["rel_pos_h"], np.float32)
    rel_pos_w = np.asarray(inputs["rel_pos_w"], np.float32)

    Wq = qkv_w[:DIM] + lb_q @ la_q
    Wk = qkv_w[DIM : 2 * DIM]
    Wv = qkv_w[2 * DIM :] + lb_v @ la_v

    wqk_host = np.ascontiguousarray(
        np.concatenate([SCALE * Wq, Wk], 0).T.astype(bf16)
    )
    wv_host = np.ascontiguousarray(Wv.T.astype(bf16))
    pw_host = np.ascontiguousarray(proj_w.T.astype(bf16))
    bqk_host = np.concatenate([SCALE * qkv_b[:DIM], qkv_b[DIM : 2 * DIM]]).astype(
        np.float32
    )
    bv_host = np.ascontiguousarray(qkv_b[2 * DIM :].astype(bf16))
    bp_host = np.ascontiguousarray(proj_b.astype(bf16))

    idx = np.arange(H)[:, None] - np.arange(H)[None, :] + (H - 1)
    Rh = rel_pos_h[idx]  # [qh, kh_j, hd]
    Rw = rel_pos_w[idx]  # [qw, kw_j, hd]
    relh_host = np.ascontiguousarray(
        (Rh / SCALE).transpose(2, 0, 1).reshape(HD, N).astype(bf16)
    )
    relw_host = np.ascontiguousarray(
        (Rw / SCALE).transpose(2, 0, 1).reshape(HD, N).astype(bf16)
    )

    kt = np.arange(N)
    oh_kh = (kt[None, :] // W == np.arange(H)[:, None]).astype(bf16)  # [14, 196]
    oh_kw = (kt[None, :] % W == np.arange(W)[:, None]).astype(bf16)
    z18 = np.zeros((18, N), bf16)
    oh_e_host = np.ascontiguousarray(np.concatenate([oh_kh, z18, oh_kw], 0))
    oh_o_host = np.ascontiguousarray(
        np.concatenate([oh_kw, z18, oh_kh, z18], 0)
    )

    shared = {
        "wqk": wqk_host,
        "wv": wv_host,
        "pw": pw_host,
        "bqk": bqk_host,
        "bv": bv_host,
        "bp": bp_host,
        "relh": relh_host,
        "relw": relw_host,
        "oh_e": oh_e_host,
        "oh_o": oh_o_host,
        "zer": z18,
    }

    x_flat = x.reshape(B_TOTAL, N, DIM)
    in_maps = []
    for c in range(NCORES):
        xc = x_flat[c * BPC : (c + 1) * BPC].reshape(BPC * N, DIM)
        xT_c = np.ascontiguousarray(xc.T.astype(bf16))
        m = dict(shared)
        m["xT"] = xT_c
        in_maps.append(m)
    return in_maps


def kernel(**inputs):
    from concourse import bass_utils

    if "nc" not in _NC_CACHE:
        _NC_CACHE["nc"] = build_module()
    nc = _NC_CACHE["nc"]
    in_maps = _host_prep(inputs)
    res = bass_utils.run_bass_kernel_spmd(
        nc, in_maps, core_ids=list(range(NCORES))
    )
    outs = [r["out"].reshape(BPC, H, W, DIM) for r in res.results]
    return np.concatenate(outs, 0)



# revision 36
# speedup vs baseline: 1.2896x; 1.2896x over previous
"""Trainium2 Bass kernel for windowed attention with LoRA + decomposed rel-pos bias.

Full-input contract: kernel(**inputs) takes the unsharded numpy inputs and
returns the full (64, 14, 14, 768) float32 output.

Strategy (8 NeuronCores, data-parallel over the 64-window batch, 8 windows/core):
  Host prep (numpy):
    - Fold LoRA into qkv weights:  Wq += lb_q@la_q, Wv += lb_v@la_v  (exact math).
    - Fold attention scale (2^-3, exact) into Wq / b_q; rel-pos tables get 1/scale.
    - Drop the k-projection bias entirely: (q+bq).(k+bk) differs from
      (q+bq).k by a per-query constant across keys, which softmax cancels.
    - Pre-transpose all weights + x; gather rel_pos tables with the (q-k)
      index map; replicate the one-hot key patterns across all 48 pair
      slots host-side so the device loads them with one big contiguous DMA.
  On chip (per core, all SBUF resident):
    - q/k projection into per-(parity, window*headpair) "augmented" tiles
      [128, 2, 48, 196]: 64 q/k rows + 14 rel-pos feature rows + 14 one-hot
      rows per parity so ONE matmul per key-chunk produces
      q@k^T*scale + rel_h + rel_w directly in PSUM (K-augmentation trick).
    - key chunks are 0:128 and 68:196 (overlapping) so every QK / v tile is
      a full 128 partitions — no junk PSUM rows, denominator contracts
      chunk0 over keys 0:68 only.
    - batched exp on ScalarE (one instruction per head over both chunks).
    - attn@v with an appended ones-column on v so the softmax denominator
      falls out of the same psum bank group; reciprocal on DVE, partition
      broadcast on the (otherwise idle) GpSimd engine, and a single fused
      multiply that normalizes while evacuating PSUM into the o2 tile.
    - rel-feature stage interleaved with the v projection so its DVE/GpSimd
      copy work overlaps TensorE matmuls; projection reads a persistent
      all-window o2 tile allocated in the space freed by the xT pool.
"""

import numpy as np
import ml_dtypes

B_TOTAL = 64
NCORES = 8
BPC = B_TOTAL // NCORES  # windows per core
H = W = 14
N = H * W  # 196 tokens per window
DIM = 768
NH = 12
HD = 64
DC = DIM // 128  # 6 contraction chunks
NKT0 = 128  # key chunk 0: keys 0:128
S1 = 68     # key chunk 1: keys 68:196 (full 128 rows, overlaps chunk 0)
SCALE = HD ** -0.5  # 0.125, exact power of two
NPR = BPC * (NH // 2)  # 48 (window, head-pair) slots per parity

# row maps inside the 128-partition augmented q/k tiles
# even parity (par=0): q/k rows 0:64, relh/kh-onehot 64:78, zeros 78:96,
#                      relw/kw-onehot 96:110; contraction range [0:110)
# odd  parity (par=1): relw/kw-onehot 0:14, zeros 14:32, relh/kh-onehot
#                      32:46, zeros 46:64, q/k rows 64:128; range [0:128)
K_EVEN = 110
K_ODD = 128

_NC_CACHE = {}


def build_module():
    from contextlib import ExitStack

    import concourse.tile as tile
    from concourse import bacc, mybir

    f32 = mybir.dt.float32
    bf16 = mybir.dt.bfloat16
    AF = mybir.ActivationFunctionType
    ALU = mybir.AluOpType

    nc = bacc.Bacc(
        "TRN2", target_bir_lowering=False, debug=False, num_devices=NCORES
    )

    T = BPC * N  # 1568 tokens per core

    xT = nc.dram_tensor("xT", [DIM, T], bf16, kind="ExternalInput").ap()
    wqk = nc.dram_tensor("wqk", [DIM, 2 * DIM], bf16, kind="ExternalInput").ap()
    wv = nc.dram_tensor("wv", [DIM, DIM], bf16, kind="ExternalInput").ap()
    pw = nc.dram_tensor("pw", [DIM, DIM], bf16, kind="ExternalInput").ap()
    bq = nc.dram_tensor("bq", [DIM], f32, kind="ExternalInput").ap()
    bv = nc.dram_tensor("bv", [DIM], bf16, kind="ExternalInput").ap()
    bp = nc.dram_tensor("bp", [DIM], f32, kind="ExternalInput").ap()
    relh = nc.dram_tensor("relh", [HD, N], bf16, kind="ExternalInput").ap()
    relw = nc.dram_tensor("relw", [HD, N], bf16, kind="ExternalInput").ap()
    # one-hot key patterns, pre-replicated across the 48 pair slots
    oh_e = nc.dram_tensor("oh_e", [46, NPR * N], bf16, kind="ExternalInput").ap()
    oh_o = nc.dram_tensor("oh_o", [64, NPR * N], bf16, kind="ExternalInput").ap()
    # feature-major output; the host transposes back to token-major
    out = nc.dram_tensor("out", [DIM, T], f32, kind="ExternalOutput").ap()

    with tile.TileContext(nc) as tc, ExitStack() as ctx:
        singles = ctx.enter_context(tc.tile_pool(name="singles", bufs=1))
        attn_pool = ctx.enter_context(tc.tile_pool(name="attn", bufs=3))
        r_pool = ctx.enter_context(tc.tile_pool(name="rp", bufs=2))
        osb_pool = ctx.enter_context(tc.tile_pool(name="osb", bufs=2))
        rd_pool = ctx.enter_context(tc.tile_pool(name="rd", bufs=2, space="DRAM"))

        # ---- resident SBUF tensors; DMA order = consumption order ----
        wqk_sb = singles.tile([128, DC, 2 * DIM], bf16)
        wqk_r = wqk.rearrange("(c p) o -> p c o", p=128)
        xT_pool_cm = tc.tile_pool(name="xt", bufs=1)
        xt_pool = xT_pool_cm.__enter__()
        xT_sb = xt_pool.tile([128, DC, T], bf16)
        xT_r = xT.rearrange("(c p) t -> p c t", p=128)
        TH = T // 2
        for c in range(DC):
            # front token-halves + q-weights first: phase 1 starts on the
            # q chunks of windows 0-3 while the rest of the inputs stream in
            nc.sync.dma_start(out=xT_sb[:, c, 0:TH], in_=xT_r[:, c, 0:TH])
            nc.sync.dma_start(out=wqk_sb[:, c, 0:DIM], in_=wqk_r[:, c, 0:DIM])
        bq_sb = singles.tile([128, DC], f32)
        nc.sync.dma_start(out=bq_sb[:], in_=bq.rearrange("(c p) -> p c", p=128))
        for c in range(DC):
            nc.sync.dma_start(
                out=wqk_sb[:, c, DIM : 2 * DIM], in_=wqk_r[:, c, DIM : 2 * DIM]
            )
        for c in range(DC):
            nc.sync.dma_start(out=xT_sb[:, c, TH:T], in_=xT_r[:, c, TH:T])
        wv_sb = singles.tile([128, DC, DIM], bf16)
        nc.sync.dma_start(out=wv_sb[:], in_=wv.rearrange("(c p) o -> p c o", p=128))
        bv_sb = singles.tile([128, DIM], bf16)
        nc.sync.dma_start(out=bv_sb[:], in_=bv.unsqueeze(0).broadcast_to([128, DIM]))
        relh_sb = singles.tile([128, N], bf16)
        nc.sync.dma_start(out=relh_sb[0:64, :], in_=relh)
        nc.sync.dma_start(out=relh_sb[64:128, :], in_=relh)
        relw_sb = singles.tile([128, N], bf16)
        nc.sync.dma_start(out=relw_sb[0:64, :], in_=relw)
        nc.sync.dma_start(out=relw_sb[64:128, :], in_=relw)

        NPAIR = 2 * NPR
        # par-major augmented tiles: [contraction, parity, (window,headpair), key]
        qaug = singles.tile([128, 2, NPR, N], bf16)
        kaug = singles.tile([128, 2, NPR, N], bf16)
        nc.sync.dma_start(
            out=kaug[64:110, 0, :, :],
            in_=oh_e.rearrange("p (s q) -> p s q", s=NPR),
        )
        nc.sync.dma_start(
            out=kaug[0:64, 1, :, :],
            in_=oh_o.rearrange("p (s q) -> p s q", s=NPR),
        )
        pw_sb = singles.tile([128, DC, DIM], bf16)
        nc.sync.dma_start(out=pw_sb[:], in_=pw.rearrange("(c p) o -> p c o", p=128))
        bp_sb = singles.tile([128, DC], f32)
        nc.sync.dma_start(out=bp_sb[:], in_=bp.rearrange("(c p) -> p c", p=128))

        # zero the feature halves of the augmented q tiles on GpSimd
        # (SBUF-only engine, idle here); rel copies later overwrite the
        # feature rows within.
        nc.gpsimd.memset(qaug[64:128, 0, :, :], 0.0)
        nc.gpsimd.memset(qaug[0:64, 1, :, :], 0.0)

        # [token-chunk rows, window, chunk, head, hd+ones]
        # chunk 0 = tokens 0:128, chunk 1 = tokens 68:196
        vall = singles.tile([128, BPC, 2, NH, HD + 1], bf16)
        nc.gpsimd.memset(vall[:, :, :, :, HD : HD + 1], 1.0)

        # rel feature stage: 4 query-rows batched per psum tile, evacuated
        # by one DVE copy each; rel_h rides inside the phase-1 q window,
        # rel_w inside the v-projection window.
        G_GRPS = [(0, 4), (4, 4), (8, 4), (12, 2)]

        def emit_relh(hx, pool):
            par = hx % 2
            q_rows = slice(0, 64) if par == 0 else slice(64, 128)
            relh_rows = slice(64, 78) if par == 0 else slice(32, 46)
            relh_tp = (0 if par == 0 else 64, 64 if par == 0 else 32)
            qslot = qaug[:, par, hx // 2 : NPR : 6, :]  # [128, BPC, N] view
            for g0, gl in G_GRPS:
                p_r = pool.tile([128, 4, 128], f32, tag="ph")
                for i in range(gl):
                    g = g0 + i
                    nc.tensor.matmul(
                        p_r[relh_rows, i, 0 : BPC * W],
                        lhsT=relh_sb[q_rows, g * W : (g + 1) * W],
                        rhs=qslot[q_rows, :, g * W : (g + 1) * W],
                        start=True,
                        stop=True,
                        tile_position=relh_tp,
                    )
                nc.vector.tensor_copy(
                    out=qslot[relh_rows, :, :].rearrange(
                        "p b (qh qw) -> p b qh qw", qh=H
                    )[:, :, g0 : g0 + gl, :],
                    in_=p_r[relh_rows, 0:gl, 0 : BPC * W].rearrange(
                        "p s (b w) -> p b s w", b=BPC
                    ),
                )

        def emit_relw(hx, pool):
            par = hx % 2
            q_rows = slice(0, 64) if par == 0 else slice(64, 128)
            relw_rows = slice(96, 110) if par == 0 else slice(0, 14)
            relw_tp = (0 if par == 0 else 64, 96 if par == 0 else 0)
            qslot = qaug[:, par, hx // 2 : NPR : 6, :]
            for g0, gl in G_GRPS:
                p_r = pool.tile([128, 4, 128], f32, tag="pw")
                for i in range(gl):
                    g = g0 + i
                    nc.tensor.matmul(
                        p_r[relw_rows, i, 0 : BPC * W],
                        lhsT=relw_sb[q_rows, g * W : (g + 1) * W],
                        rhs=qslot[q_rows, :, g : g + 13 * W + 1 : W],
                        start=True,
                        stop=True,
                        tile_position=relw_tp,
                    )
                # ScalarE is idle during the v projection; use it here so
                # DVE keeps its headroom for the vall bias-adds
                nc.scalar.activation(
                    out=qslot[relw_rows, :, :].rearrange(
                        "p b (qh qw) -> p b qh qw", qh=H
                    )[:, :, :, g0 : g0 + gl],
                    in_=p_r[relw_rows, 0:gl, 0 : BPC * W].rearrange(
                        "p s (b q) -> p b q s", b=BPC
                    ),
                    func=AF.Copy,
                    scale=1.0,
                )

        # ---- phase 1: q/k projection ----
        # pool entry order is LIFO-release
        p2_cm = tc.tile_pool(name="p2", bufs=2, space="PSUM")
        p2_pool = p2_cm.__enter__()
        pq_cm = tc.tile_pool(name="pq", bufs=4, space="PSUM")
        pq_pool = pq_cm.__enter__()

        # q chunks, window-pair major (matches the DMA arrival order)
        for b2 in range(BPC // 2):
            tok = slice(2 * b2 * N, (2 * b2 + 2) * N)
            for oc in range(DC):
                p_q = pq_pool.tile([128, 512], f32, tag="pq")
                for dc in range(DC):
                    nc.tensor.matmul(
                        p_q[:, 0 : 2 * N],
                        lhsT=wqk_sb[:, dc, oc * 128 : (oc + 1) * 128],
                        rhs=xT_sb[:, dc, tok],
                        start=(dc == 0),
                        stop=(dc == DC - 1),
                    )
                hh = oc  # head-pair index; heads (2*oc, 2*oc+1)
                for par in range(2):
                    rows = slice(0, 64) if par == 0 else slice(64, 128)
                    nc.scalar.activation(
                        out=qaug[rows, par, 2 * b2 * 6 + hh : 2 * b2 * 6 + hh + 7 : 6, :],
                        in_=p_q[rows, 0 : 2 * N].rearrange("p (w q) -> p w q", w=2),
                        func=AF.Identity,
                        bias=bq_sb[rows, oc : oc + 1],
                        scale=1.0,
                    )

        pq_cm.__exit__(None, None, None)
        psdh_cm = tc.tile_pool(name="psdh", bufs=2, space="PSUM")
        psdh_pool = psdh_cm.__enter__()

        # k chunks (oc-pairs share a 2-bank psum tile, batched copy, no
        # bias), with the rel_h stage interleaved: its DVE copies overlap
        # the k matmuls
        for kk in range(DC // 2):
            for b2 in range(BPC // 2):
                tok = slice(2 * b2 * N, (2 * b2 + 2) * N)
                p_2 = p2_pool.tile([128, 2, 512], f32, tag="p2")
                for s in range(2):
                    oc = DC + 2 * kk + s
                    for dc in range(DC):
                        nc.tensor.matmul(
                            p_2[:, s, 0 : 2 * N],
                            lhsT=wqk_sb[:, dc, oc * 128 : (oc + 1) * 128],
                            rhs=xT_sb[:, dc, tok],
                            start=(dc == 0),
                            stop=(dc == DC - 1),
                        )
                for par in range(2):
                    rows = slice(0, 64) if par == 0 else slice(64, 128)
                    nc.scalar.activation(
                        out=kaug[rows, par, :, :]
                        .rearrange("p (w hh) q -> p w hh q", w=BPC)[
                            :, 2 * b2 : 2 * b2 + 2, 2 * kk : 2 * kk + 2, :
                        ],
                        in_=p_2[rows, :, 0 : 2 * N].rearrange(
                            "p s (w q) -> p w s q", w=2
                        ),
                        func=AF.Copy,
                        scale=1.0,
                    )
            for hx in range(4 * kk, 4 * kk + 4):
                emit_relh(hx, psdh_pool)

        psdh_cm.__exit__(None, None, None)

        # ---- phase 1b: v projection, interleaved with rel_w ----
        psdw_cm = tc.tile_pool(name="psdw", bufs=2, space="PSUM")
        psdw_pool = psdw_cm.__enter__()
        for b in range(BPC):
            for i in range(2):  # token chunks 0:128 / 68:196
                t0 = b * N + (0 if i == 0 else S1)
                p_v = p2_pool.tile([128, 2, 512], f32, tag="p2")
                for half in range(2):
                    for dc in range(DC):
                        nc.tensor.matmul(
                            p_v[:, half, 0:384],
                            lhsT=xT_sb[:, dc, t0 : t0 + 128],
                            rhs=wv_sb[:, dc, half * 384 : (half + 1) * 384],
                            start=(dc == 0),
                            stop=(dc == DC - 1),
                        )
                nc.vector.tensor_tensor(
                    out=vall[:, b, i, :, 0:HD].rearrange(
                        "p (s h) d -> p s h d", s=2
                    ),
                    in0=p_v[:, :, 0:384].rearrange("p s (h d) -> p s h d", h=6),
                    in1=bv_sb[:, :].rearrange("p (s h d) -> p s h d", s=2, h=6),
                    op=ALU.add,
                )
            if b >= 1 and b <= 6:
                emit_relw(2 * (b - 1), psdw_pool)
                emit_relw(2 * (b - 1) + 1, psdw_pool)

        # xT no longer needed; free its zone for o2_all (LIFO release)
        psdw_cm.__exit__(None, None, None)
        p2_cm.__exit__(None, None, None)
        xT_pool_cm.__exit__(None, None, None)
        o2_pool = ctx.enter_context(tc.tile_pool(name="o2", bufs=1))
        o2_all = o2_pool.tile([128, DC, T], bf16)

        # ---- phases 2+3, head-pair major attention ----
        pa_cm = tc.tile_pool(name="pa", bufs=2, space="PSUM")
        pa_pool = pa_cm.__enter__()
        po_cm = tc.tile_pool(name="po", bufs=2, space="PSUM")
        po_pool = po_cm.__enter__()
        pd_cm = tc.tile_pool(name="pd", bufs=2, space="PSUM")
        pd_pool = pd_cm.__enter__()

        def emit_qk(b, hx, p_a):
            par = hx % 2
            hh = hx // 2
            slot = b * 6 + hh
            krange = slice(0, K_EVEN) if par == 0 else slice(0, K_ODD)
            nc.tensor.matmul(
                p_a[:, par, 0, 0:N],
                lhsT=kaug[krange, par, slot, 0:NKT0],
                rhs=qaug[krange, par, slot, :],
                start=True,
                stop=True,
            )
            nc.tensor.matmul(
                p_a[:, par, 1, 0:N],
                lhsT=kaug[krange, par, slot, S1:N],
                rhs=qaug[krange, par, slot, :],
                start=True,
                stop=True,
            )

        def emit_denom(b, hx, a_sb, pdf, first, last):
            # one accumulation group: row 0, even head at cols 0:N, odd head
            # at cols N:2N (start=True pre-zeroes the whole row-0 region, the
            # later matmuls accumulate onto pending-zero bytes)
            par = hx % 2
            nc.tensor.matmul(
                pdf[0:1, par * N : (par + 1) * N],
                lhsT=vall[0:S1, b, 0, hx, HD : HD + 1],
                rhs=a_sb[0:S1, par, 0, :],
                start=first,
                stop=False,
                tile_position=(0, 0),
                skip_group_check=True,
            )
            nc.tensor.matmul(
                pdf[0:1, par * N : (par + 1) * N],
                lhsT=vall[:, b, 1, hx, HD : HD + 1],
                rhs=a_sb[:, par, 1, :],
                start=False,
                stop=last,
                tile_position=(0, 0),
                skip_group_check=True,
            )

        def emit_av(b, hx, a_sb, p_o):
            par = hx % 2
            rows = slice(0, 64) if par == 0 else slice(64, 128)
            av_tp = (0, 0) if par == 0 else (0, 64)
            nc.tensor.matmul(
                p_o[rows, 0:N],
                lhsT=vall[0:S1, b, 0, hx, 0:HD],
                rhs=a_sb[0:S1, par, 0, :],
                start=True,
                stop=False,
                tile_position=av_tp,
                skip_group_check=True,
            )
            nc.tensor.matmul(
                p_o[rows, 0:N],
                lhsT=vall[:, b, 1, hx, 0:HD],
                rhs=a_sb[:, par, 1, :],
                start=False,
                stop=True,
                tile_position=av_tp,
                skip_group_check=True,
            )

        for hh in range(NH // 2):
            h0, h1 = 2 * hh, 2 * hh + 1
            r_hh = r_pool.tile([1, BPC, 2, N], bf16, tag="rw")
            rb_hh = r_pool.tile([128, BPC, N], bf16, tag="rb")

            def flush(b, a_sb, r_hh=r_hh, rb_hh=rb_hh, hh=hh, h0=h0, h1=h1):
                """Post-exp work for window b: denoms, AV, recip, bcast, mult."""
                p_dd = pd_pool.tile([128, 4, 128], f32, tag="pd")
                pdf = p_dd.rearrange("p s c -> p (s c)")
                emit_denom(b, h0, a_sb, pdf, True, False)
                emit_denom(b, h1, a_sb, pdf, False, True)
                p_o = po_pool.tile([128, 512], f32, tag="po")
                emit_av(b, h0, a_sb, p_o)
                emit_av(b, h1, a_sb, p_o)
                with nc.allow_low_precision(reason="bf16 softmax recip"):
                    nc.vector.reciprocal(
                        out=r_hh[0:1, b, :, :], in_=pdf[0:1, 0 : 2 * N]
                    )
                # evacuate PSUM unnormalized; the batched normalize happens
                # after the per-head broadcast bounce below
                nc.vector.tensor_copy(
                    out=o2_all[:, hh, b * N : (b + 1) * N],
                    in_=p_o[:, 0:N],
                )

            pend = None
            for b in range(BPC):
                # both parities of the head-pair share one 2-bank psum tile
                # so a single exp instruction covers all four QK chunks;
                # emission runs one window ahead of the post-exp work so the
                # PE queue never stalls behind ScalarE.
                p_a = pa_pool.tile([128, 2, 2, 256], f32, tag="pa")
                a_sb = attn_pool.tile([128, 2, 2, N], bf16, tag="a")
                emit_qk(b, h0, p_a)
                emit_qk(b, h1, p_a)
                nc.scalar.activation(
                    out=a_sb[:], in_=p_a[:, :, :, 0:N], func=AF.Exp, scale=1.0
                )
                if pend is not None:
                    flush(*pend)
                pend = (b, a_sb)
            flush(*pend)

            # partition-broadcast the reciprocals with a DRAM bounce, then
            # normalize o2 in place on the (otherwise idle, SBUF-only)
            # GpSimd engine
            dd = rd_pool.tile([2, BPC, N], bf16, tag="rd")
            nc.sync.dma_start(out=dd[0:1, :, :], in_=r_hh[0:1, :, 0, :])
            nc.sync.dma_start(out=dd[1:2, :, :], in_=r_hh[0:1, :, 1, :])
            nc.sync.dma_start(
                out=rb_hh[0:64, :, :], in_=dd[0:1, :, :].broadcast_to([64, BPC, N])
            )
            nc.sync.dma_start(
                out=rb_hh[64:128, :, :],
                in_=dd[1:2, :, :].broadcast_to([64, BPC, N]),
            )
            nc.gpsimd.tensor_tensor(
                out=o2_all[:, hh, :].rearrange("p (b q) -> p b q", b=BPC),
                in0=o2_all[:, hh, :].rearrange("p (b q) -> p b q", b=BPC),
                in1=rb_hh[:, :, :],
                op=ALU.mult,
            )

        pd_cm.__exit__(None, None, None)
        po_cm.__exit__(None, None, None)
        pa_cm.__exit__(None, None, None)

        # ---- phase 4: projection, feature-major output ----
        # lhsT = proj weights (stationary), rhs = o2 token stream; the bias
        # is per-partition here so ScalarE adds it while evacuating PSUM.
        pp_cm = tc.tile_pool(name="pp", bufs=3, space="PSUM")
        pp_pool = pp_cm.__enter__()
        TCH = [(0, 512), (512, 512), (1024, 512), (1536, 32)]
        for fc in range(DC):
            o_sb = osb_pool.tile([128, T], f32, tag="osb")
            for t0, tn in TCH:
                p_p = pp_pool.tile([128, 512], f32, tag="pp")
                for cc in range(DC):
                    nc.tensor.matmul(
                        p_p[:, 0:tn],
                        lhsT=pw_sb[:, cc, fc * 128 : (fc + 1) * 128],
                        rhs=o2_all[:, cc, t0 : t0 + tn],
                        start=(cc == 0),
                        stop=(cc == DC - 1),
                    )
                nc.scalar.activation(
                    out=o_sb[:, t0 : t0 + tn],
                    in_=p_p[:, 0:tn],
                    func=AF.Identity,
                    bias=bp_sb[:, fc : fc + 1],
                    scale=1.0,
                )
                if t0 + tn in (1024, T):
                    # split the writeback so the last-chunk drain is short
                    w0 = 0 if t0 + tn == 1024 else 1024
                    nc.sync.dma_start(
                        out=out[fc * 128 : (fc + 1) * 128, w0 : t0 + tn],
                        in_=o_sb[:, w0 : t0 + tn],
                    )
        pp_cm.__exit__(None, None, None)

    nc.finalize()
    return nc


def _host_prep(inputs):
    bf16 = ml_dtypes.bfloat16
    x = np.asarray(inputs["x"], np.float32)
    qkv_w = np.asarray(inputs["qkv_w"], np.float32)
    qkv_b = np.asarray(inputs["qkv_b"], np.float32)
    proj_w = np.asarray(inputs["proj_w"], np.float32)
    proj_b = np.asarray(inputs["proj_b"], np.float32)
    la_q = np.asarray(inputs["la_q"], np.float32)
    lb_q = np.asarray(inputs["lb_q"], np.float32)
    la_v = np.asarray(inputs["la_v"], np.float32)
    lb_v = np.asarray(inputs["lb_v"], np.float32)
    rel_pos_h = np.asarray(inputs["rel_pos_h"], np.float32)
    rel_pos_w = np.asarray(inputs["rel_pos_w"], np.float32)

    Wq = qkv_w[:DIM] + lb_q @ la_q
    Wk = qkv_w[DIM : 2 * DIM]
    Wv = qkv_w[2 * DIM :] + lb_v @ la_v

    wqk_host = np.ascontiguousarray(
        np.concatenate([SCALE * Wq, Wk], 0).T.astype(bf16)
    )
    wv_host = np.ascontiguousarray(Wv.T.astype(bf16))
    pw_host = np.ascontiguousarray(proj_w.T.astype(bf16))
    bq_host = (SCALE * qkv_b[:DIM]).astype(np.float32)
    bv_host = np.ascontiguousarray(qkv_b[2 * DIM :].astype(bf16))
    bp_host = proj_b.astype(np.float32)

    idx = np.arange(H)[:, None] - np.arange(H)[None, :] + (H - 1)
    Rh = rel_pos_h[idx]  # [qh, kh_j, hd]
    Rw = rel_pos_w[idx]  # [qw, kw_j, hd]
    relh_host = np.ascontiguousarray(
        (Rh / SCALE).transpose(2, 0, 1).reshape(HD, N).astype(bf16)
    )
    relw_host = np.ascontiguousarray(
        (Rw / SCALE).transpose(2, 0, 1).reshape(HD, N).astype(bf16)
    )

    kt = np.arange(N)
    oh_kh = (kt[None, :] // W == np.arange(H)[:, None]).astype(bf16)  # [14, 196]
    oh_kw = (kt[None, :] % W == np.arange(W)[:, None]).astype(bf16)
    z18 = np.zeros((18, N), bf16)
    oh_e_1 = np.concatenate([oh_kh, z18, oh_kw], 0)  # [46, 196]
    oh_o_1 = np.concatenate([oh_kw, z18, oh_kh, z18], 0)  # [64, 196]
    # replicate across the 48 (window, head-pair) slots -> contiguous DMA
    oh_e_host = np.ascontiguousarray(
        np.broadcast_to(oh_e_1[:, None, :], (46, NPR, N)).reshape(46, NPR * N)
    )
    oh_o_host = np.ascontiguousarray(
        np.broadcast_to(oh_o_1[:, None, :], (64, NPR, N)).reshape(64, NPR * N)
    )

    shared = {
        "wqk": wqk_host,
        "wv": wv_host,
        "pw": pw_host,
        "bq": bq_host,
        "bv": bv_host,
        "bp": bp_host,
        "relh": relh_host,
        "relw": relw_host,
        "oh_e": oh_e_host,
        "oh_o": oh_o_host,
    }

    x_flat = x.reshape(B_TOTAL, N, DIM)
    in_maps = []
    for c in range(NCORES):
        xc = x_flat[c * BPC : (c + 1) * BPC].reshape(BPC * N, DIM)
        xT_c = np.ascontiguousarray(xc.T.astype(bf16))
        m = dict(shared)
        m["xT"] = xT_c
        in_maps.append(m)
    return in_maps


def kernel(**inputs):
    from concourse import bass_utils

    if "nc" not in _NC_CACHE:
        _NC_CACHE["nc"] = build_module()
    nc = _NC_CACHE["nc"]
    in_maps = _host_prep(inputs)
    res = bass_utils.run_bass_kernel_spmd(
        nc, in_maps, core_ids=list(range(NCORES))
    )
    outs = [
        np.ascontiguousarray(r["out"].T).reshape(BPC, H, W, DIM)
        for r in res.results
    ]
    return np.concatenate(outs, 0)


# revision 38
# speedup vs baseline: 1.3141x; 1.0190x over previous
"""Trainium2 Bass kernel for windowed attention with LoRA + decomposed rel-pos bias.

Full-input contract: kernel(**inputs) takes the unsharded numpy inputs and
returns the full (64, 14, 14, 768) float32 output.

Strategy (8 NeuronCores, data-parallel over the 64-window batch, 8 windows/core):
  Host prep (numpy):
    - Fold LoRA into qkv weights:  Wq += lb_q@la_q, Wv += lb_v@la_v  (exact math).
    - Fold attention scale (2^-3, exact) into Wq / b_q; rel-pos tables get 1/scale.
    - Drop the k-projection bias entirely: (q+bq).(k+bk) differs from
      (q+bq).k by a per-query constant across keys, which softmax cancels.
    - Pre-transpose all weights + x; gather rel_pos tables with the (q-k)
      index map; replicate the one-hot key patterns across all 48 pair
      slots host-side so the device loads them with one big contiguous DMA.
  On chip (per core, all SBUF resident):
    - q/k projection into per-(parity, window*headpair) "augmented" tiles
      [128, 2, 48, 196]: 64 q/k rows + 14 rel-pos feature rows + 14 one-hot
      rows per parity so ONE matmul per key-chunk produces
      q@k^T*scale + rel_h + rel_w directly in PSUM (K-augmentation trick).
    - key chunks are 0:128 and 68:196 (overlapping) so every QK / v tile is
      a full 128 partitions — no junk PSUM rows, denominator contracts
      chunk0 over keys 0:68 only.
    - batched exp on ScalarE (one instruction per head over both chunks).
    - attn@v with an appended ones-column on v so the softmax denominator
      falls out of the same psum bank group; reciprocal on DVE, partition
      broadcast on the (otherwise idle) GpSimd engine, and a single fused
      multiply that normalizes while evacuating PSUM into the o2 tile.
    - rel-feature stage interleaved with the v projection so its DVE/GpSimd
      copy work overlaps TensorE matmuls; projection reads a persistent
      all-window o2 tile allocated in the space freed by the xT pool.
"""

import numpy as np
import ml_dtypes

B_TOTAL = 64
NCORES = 8
BPC = B_TOTAL // NCORES  # windows per core
H = W = 14
N = H * W  # 196 tokens per window
DIM = 768
NH = 12
HD = 64
DC = DIM // 128  # 6 contraction chunks
NKT0 = 128  # key chunk 0: keys 0:128
S1 = 68     # key chunk 1: keys 68:196 (full 128 rows, overlaps chunk 0)
SCALE = HD ** -0.5  # 0.125, exact power of two
NPR = BPC * (NH // 2)  # 48 (window, head-pair) slots per parity

# row maps inside the 128-partition augmented q/k tiles
# even parity (par=0): q/k rows 0:64, relh/kh-onehot 64:78, zeros 78:96,
#                      relw/kw-onehot 96:110; contraction range [0:110)
# odd  parity (par=1): relw/kw-onehot 0:14, zeros 14:32, relh/kh-onehot
#                      32:46, zeros 46:64, q/k rows 64:128; range [0:128)
K_EVEN = 110
K_ODD = 128

_NC_CACHE = {}


def build_module():
    from contextlib import ExitStack

    import concourse.tile as tile
    from concourse import bacc, mybir

    f32 = mybir.dt.float32
    bf16 = mybir.dt.bfloat16
    AF = mybir.ActivationFunctionType
    ALU = mybir.AluOpType

    nc = bacc.Bacc(
        "TRN2", target_bir_lowering=False, debug=False, num_devices=NCORES
    )

    T = BPC * N  # 1568 tokens per core

    xT = nc.dram_tensor("xT", [DIM, T], bf16, kind="ExternalInput").ap()
    wqk = nc.dram_tensor("wqk", [DIM, 2 * DIM], bf16, kind="ExternalInput").ap()
    wv = nc.dram_tensor("wv", [DIM, DIM], bf16, kind="ExternalInput").ap()
    pw = nc.dram_tensor("pw", [DIM, DIM], bf16, kind="ExternalInput").ap()
    bq = nc.dram_tensor("bq", [DIM], f32, kind="ExternalInput").ap()
    bv = nc.dram_tensor("bv", [DIM], bf16, kind="ExternalInput").ap()
    bp = nc.dram_tensor("bp", [DIM], f32, kind="ExternalInput").ap()
    relh = nc.dram_tensor("relh", [HD, N], bf16, kind="ExternalInput").ap()
    relw = nc.dram_tensor("relw", [HD, N], bf16, kind="ExternalInput").ap()
    # one-hot key patterns, pre-replicated across the 48 pair slots
    oh_e = nc.dram_tensor("oh_e", [46, NPR * N], bf16, kind="ExternalInput").ap()
    oh_o = nc.dram_tensor("oh_o", [64, NPR * N], bf16, kind="ExternalInput").ap()
    # feature-major output; the host transposes back to token-major
    out = nc.dram_tensor("out", [DIM, T], f32, kind="ExternalOutput").ap()

    with tile.TileContext(nc) as tc, ExitStack() as ctx:
        singles = ctx.enter_context(tc.tile_pool(name="singles", bufs=1))
        attn_pool = ctx.enter_context(tc.tile_pool(name="attn", bufs=3))
        r_pool = ctx.enter_context(tc.tile_pool(name="rp", bufs=2))
        osb_pool = ctx.enter_context(tc.tile_pool(name="osb", bufs=2))
        rd_pool = ctx.enter_context(tc.tile_pool(name="rd", bufs=2, space="DRAM"))

        # ---- resident SBUF tensors; DMA order = consumption order ----
        wqk_sb = singles.tile([128, DC, 2 * DIM], bf16)
        wqk_r = wqk.rearrange("(c p) o -> p c o", p=128)
        xT_pool_cm = tc.tile_pool(name="xt", bufs=1)
        xt_pool = xT_pool_cm.__enter__()
        xT_sb = xt_pool.tile([128, DC, T], bf16)
        xT_r = xT.rearrange("(c p) t -> p c t", p=128)
        TH = T // 2
        for c in range(DC):
            # front token-halves + q-weights first: phase 1 starts on the
            # q chunks of windows 0-3 while the rest of the inputs stream in
            nc.sync.dma_start(out=xT_sb[:, c, 0:TH], in_=xT_r[:, c, 0:TH])
            nc.sync.dma_start(out=wqk_sb[:, c, 0:DIM], in_=wqk_r[:, c, 0:DIM])
        bq_sb = singles.tile([128, DC], f32)
        nc.sync.dma_start(out=bq_sb[:], in_=bq.rearrange("(c p) -> p c", p=128))
        for c in range(DC):
            nc.sync.dma_start(
                out=wqk_sb[:, c, DIM : 2 * DIM], in_=wqk_r[:, c, DIM : 2 * DIM]
            )
        for c in range(DC):
            nc.sync.dma_start(out=xT_sb[:, c, TH:T], in_=xT_r[:, c, TH:T])
        wv_sb = singles.tile([128, DC, DIM], bf16)
        nc.sync.dma_start(out=wv_sb[:], in_=wv.rearrange("(c p) o -> p c o", p=128))
        bv_sb = singles.tile([128, DIM], bf16)
        nc.sync.dma_start(out=bv_sb[:], in_=bv.unsqueeze(0).broadcast_to([128, DIM]))
        relh_sb = singles.tile([128, N], bf16)
        nc.sync.dma_start(out=relh_sb[0:64, :], in_=relh)
        nc.sync.dma_start(out=relh_sb[64:128, :], in_=relh)
        relw_sb = singles.tile([128, N], bf16)
        nc.sync.dma_start(out=relw_sb[0:64, :], in_=relw)
        nc.sync.dma_start(out=relw_sb[64:128, :], in_=relw)

        NPAIR = 2 * NPR
        # par-major augmented tiles: [contraction, parity, (window,headpair), key]
        qaug = singles.tile([128, 2, NPR, N], bf16)
        kaug = singles.tile([128, 2, NPR, N], bf16)
        nc.sync.dma_start(
            out=kaug[64:110, 0, :, :],
            in_=oh_e.rearrange("p (s q) -> p s q", s=NPR),
        )
        nc.sync.dma_start(
            out=kaug[0:64, 1, :, :],
            in_=oh_o.rearrange("p (s q) -> p s q", s=NPR),
        )
        pw_sb = singles.tile([128, DC, DIM], bf16)
        nc.sync.dma_start(out=pw_sb[:], in_=pw.rearrange("(c p) o -> p c o", p=128))
        bp_sb = singles.tile([128, DC], f32)
        nc.sync.dma_start(out=bp_sb[:], in_=bp.rearrange("(c p) -> p c", p=128))

        # zero the feature halves of the augmented q tiles on GpSimd
        # (SBUF-only engine, idle here); rel copies later overwrite the
        # feature rows within.
        nc.gpsimd.memset(qaug[64:128, 0, :, :], 0.0)
        nc.gpsimd.memset(qaug[0:64, 1, :, :], 0.0)

        # [token-chunk rows, window, chunk, head, hd+ones]
        # chunk 0 = tokens 0:128, chunk 1 = tokens 68:196
        vall = singles.tile([128, BPC, 2, NH, HD + 1], bf16)
        nc.gpsimd.memset(vall[:, :, :, :, HD : HD + 1], 1.0)

        # rel feature stage: 4 query-rows batched per psum tile, evacuated
        # by one DVE copy each; rel_h rides inside the phase-1 q window,
        # rel_w inside the v-projection window.
        G_GRPS = [(0, 4), (4, 4), (8, 4), (12, 2)]

        def emit_relh(hx, pool):
            par = hx % 2
            q_rows = slice(0, 64) if par == 0 else slice(64, 128)
            relh_rows = slice(64, 78) if par == 0 else slice(32, 46)
            relh_tp = (0 if par == 0 else 64, 64 if par == 0 else 32)
            qslot = qaug[:, par, hx // 2 : NPR : 6, :]  # [128, BPC, N] view
            for g0, gl in G_GRPS:
                p_r = pool.tile([128, 4, 128], f32, tag="ph")
                for i in range(gl):
                    g = g0 + i
                    nc.tensor.matmul(
                        p_r[relh_rows, i, 0 : BPC * W],
                        lhsT=relh_sb[q_rows, g * W : (g + 1) * W],
                        rhs=qslot[q_rows, :, g * W : (g + 1) * W],
                        start=True,
                        stop=True,
                        tile_position=relh_tp,
                    )
                nc.vector.tensor_copy(
                    out=qslot[relh_rows, :, :].rearrange(
                        "p b (qh qw) -> p b qh qw", qh=H
                    )[:, :, g0 : g0 + gl, :],
                    in_=p_r[relh_rows, 0:gl, 0 : BPC * W].rearrange(
                        "p s (b w) -> p b s w", b=BPC
                    ),
                )

        def emit_relw(hx, pool):
            par = hx % 2
            q_rows = slice(0, 64) if par == 0 else slice(64, 128)
            relw_rows = slice(96, 110) if par == 0 else slice(0, 14)
            relw_tp = (0 if par == 0 else 64, 96 if par == 0 else 0)
            qslot = qaug[:, par, hx // 2 : NPR : 6, :]
            for g0, gl in G_GRPS:
                p_r = pool.tile([128, 4, 128], f32, tag="pw")
                for i in range(gl):
                    g = g0 + i
                    nc.tensor.matmul(
                        p_r[relw_rows, i, 0 : BPC * W],
                        lhsT=relw_sb[q_rows, g * W : (g + 1) * W],
                        rhs=qslot[q_rows, :, g : g + 13 * W + 1 : W],
                        start=True,
                        stop=True,
                        tile_position=relw_tp,
                    )
                # ScalarE is idle during the v projection; use it here so
                # DVE keeps its headroom for the vall bias-adds
                nc.scalar.activation(
                    out=qslot[relw_rows, :, :].rearrange(
                        "p b (qh qw) -> p b qh qw", qh=H
                    )[:, :, :, g0 : g0 + gl],
                    in_=p_r[relw_rows, 0:gl, 0 : BPC * W].rearrange(
                        "p s (b q) -> p b q s", b=BPC
                    ),
                    func=AF.Copy,
                    scale=1.0,
                )

        # ---- phase 1: q/k projection ----
        # pool entry order is LIFO-release
        p2_cm = tc.tile_pool(name="p2", bufs=2, space="PSUM")
        p2_pool = p2_cm.__enter__()
        pq_cm = tc.tile_pool(name="pq", bufs=4, space="PSUM")
        pq_pool = pq_cm.__enter__()

        # q chunks, window-pair major (matches the DMA arrival order)
        for b2 in range(BPC // 2):
            tok = slice(2 * b2 * N, (2 * b2 + 2) * N)
            for oc in range(DC):
                p_q = pq_pool.tile([128, 512], f32, tag="pq")
                for dc in range(DC):
                    nc.tensor.matmul(
                        p_q[:, 0 : 2 * N],
                        lhsT=wqk_sb[:, dc, oc * 128 : (oc + 1) * 128],
                        rhs=xT_sb[:, dc, tok],
                        start=(dc == 0),
                        stop=(dc == DC - 1),
                    )
                hh = oc  # head-pair index; heads (2*oc, 2*oc+1)
                for par in range(2):
                    rows = slice(0, 64) if par == 0 else slice(64, 128)
                    nc.scalar.activation(
                        out=qaug[rows, par, 2 * b2 * 6 + hh : 2 * b2 * 6 + hh + 7 : 6, :],
                        in_=p_q[rows, 0 : 2 * N].rearrange("p (w q) -> p w q", w=2),
                        func=AF.Identity,
                        bias=bq_sb[rows, oc : oc + 1],
                        scale=1.0,
                    )

        pq_cm.__exit__(None, None, None)
        psdh_cm = tc.tile_pool(name="psdh", bufs=2, space="PSUM")
        psdh_pool = psdh_cm.__enter__()

        # k chunks (oc-pairs share a 2-bank psum tile, batched copy, no
        # bias), with the rel_h stage interleaved: its DVE copies overlap
        # the k matmuls
        for kk in range(DC // 2):
            for b2 in range(BPC // 2):
                tok = slice(2 * b2 * N, (2 * b2 + 2) * N)
                p_2 = p2_pool.tile([128, 2, 512], f32, tag="p2")
                for s in range(2):
                    oc = DC + 2 * kk + s
                    for dc in range(DC):
                        nc.tensor.matmul(
                            p_2[:, s, 0 : 2 * N],
                            lhsT=wqk_sb[:, dc, oc * 128 : (oc + 1) * 128],
                            rhs=xT_sb[:, dc, tok],
                            start=(dc == 0),
                            stop=(dc == DC - 1),
                        )
                for par in range(2):
                    rows = slice(0, 64) if par == 0 else slice(64, 128)
                    nc.scalar.activation(
                        out=kaug[rows, par, :, :]
                        .rearrange("p (w hh) q -> p w hh q", w=BPC)[
                            :, 2 * b2 : 2 * b2 + 2, 2 * kk : 2 * kk + 2, :
                        ],
                        in_=p_2[rows, :, 0 : 2 * N].rearrange(
                            "p s (w q) -> p w s q", w=2
                        ),
                        func=AF.Copy,
                        scale=1.0,
                    )
            for hx in range(4 * kk, 4 * kk + 4):
                emit_relh(hx, psdh_pool)

        psdh_cm.__exit__(None, None, None)

        # ---- phase 1b: v projection, interleaved with rel_w ----
        psdw_cm = tc.tile_pool(name="psdw", bufs=2, space="PSUM")
        psdw_pool = psdw_cm.__enter__()
        for b in range(BPC):
            for i in range(2):  # token chunks 0:128 / 68:196
                t0 = b * N + (0 if i == 0 else S1)
                p_v = p2_pool.tile([128, 2, 512], f32, tag="p2")
                for half in range(2):
                    for dc in range(DC):
                        nc.tensor.matmul(
                            p_v[:, half, 0:384],
                            lhsT=xT_sb[:, dc, t0 : t0 + 128],
                            rhs=wv_sb[:, dc, half * 384 : (half + 1) * 384],
                            start=(dc == 0),
                            stop=(dc == DC - 1),
                        )
                nc.vector.tensor_tensor(
                    out=vall[:, b, i, :, 0:HD].rearrange(
                        "p (s h) d -> p s h d", s=2
                    ),
                    in0=p_v[:, :, 0:384].rearrange("p s (h d) -> p s h d", h=6),
                    in1=bv_sb[:, :].rearrange("p (s h d) -> p s h d", s=2, h=6),
                    op=ALU.add,
                )
            if b >= 1 and b <= 6:
                emit_relw(2 * (b - 1), psdw_pool)
                emit_relw(2 * (b - 1) + 1, psdw_pool)

        # xT no longer needed; free its zone for o2_all (LIFO release)
        psdw_cm.__exit__(None, None, None)
        p2_cm.__exit__(None, None, None)
        xT_pool_cm.__exit__(None, None, None)
        o2_pool = ctx.enter_context(tc.tile_pool(name="o2", bufs=1))
        o2_all = o2_pool.tile([128, DC, T], bf16)

        # ---- phases 2+3, head-pair major attention ----
        pa_cm = tc.tile_pool(name="pa", bufs=2, space="PSUM")
        pa_pool = pa_cm.__enter__()
        po_cm = tc.tile_pool(name="po", bufs=2, space="PSUM")
        po_pool = po_cm.__enter__()
        pd_cm = tc.tile_pool(name="pd", bufs=2, space="PSUM")
        pd_pool = pd_cm.__enter__()

        def emit_qk(b, hx, p_a):
            par = hx % 2
            hh = hx // 2
            slot = b * 6 + hh
            krange = slice(0, K_EVEN) if par == 0 else slice(0, K_ODD)
            nc.tensor.matmul(
                p_a[:, par, 0, 0:N],
                lhsT=kaug[krange, par, slot, 0:NKT0],
                rhs=qaug[krange, par, slot, :],
                start=True,
                stop=True,
            )
            nc.tensor.matmul(
                p_a[:, par, 1, 0:N],
                lhsT=kaug[krange, par, slot, S1:N],
                rhs=qaug[krange, par, slot, :],
                start=True,
                stop=True,
            )

        def emit_denom(b, hx, a_sb, pdf, first, last):
            # one accumulation group: row 0, even head at cols 0:N, odd head
            # at cols N:2N (start=True pre-zeroes the whole row-0 region, the
            # later matmuls accumulate onto pending-zero bytes)
            par = hx % 2
            nc.tensor.matmul(
                pdf[0:1, par * N : (par + 1) * N],
                lhsT=vall[0:S1, b, 0, hx, HD : HD + 1],
                rhs=a_sb[0:S1, par, 0, :],
                start=first,
                stop=False,
                tile_position=(0, 0),
                skip_group_check=True,
            )
            nc.tensor.matmul(
                pdf[0:1, par * N : (par + 1) * N],
                lhsT=vall[:, b, 1, hx, HD : HD + 1],
                rhs=a_sb[:, par, 1, :],
                start=False,
                stop=last,
                tile_position=(0, 0),
                skip_group_check=True,
            )

        def emit_av(b, hx, a_sb, p_o):
            par = hx % 2
            rows = slice(0, 64) if par == 0 else slice(64, 128)
            av_tp = (0, 0) if par == 0 else (0, 64)
            nc.tensor.matmul(
                p_o[rows, 0:N],
                lhsT=vall[0:S1, b, 0, hx, 0:HD],
                rhs=a_sb[0:S1, par, 0, :],
                start=True,
                stop=False,
                tile_position=av_tp,
                skip_group_check=True,
            )
            nc.tensor.matmul(
                p_o[rows, 0:N],
                lhsT=vall[:, b, 1, hx, 0:HD],
                rhs=a_sb[:, par, 1, :],
                start=False,
                stop=True,
                tile_position=av_tp,
                skip_group_check=True,
            )

        for hh in range(NH // 2):
            h0, h1 = 2 * hh, 2 * hh + 1
            r_hh = r_pool.tile([1, BPC, 2, N], bf16, tag="rw")
            rb_hh = r_pool.tile([128, BPC, N], bf16, tag="rb")

            def flush(b, a_sb, r_hh=r_hh, rb_hh=rb_hh, hh=hh, h0=h0, h1=h1):
                """Post-exp work for window b: denoms, AV, recip, bcast, mult."""
                p_dd = pd_pool.tile([128, 4, 128], f32, tag="pd")
                pdf = p_dd.rearrange("p s c -> p (s c)")
                emit_denom(b, h0, a_sb, pdf, True, False)
                emit_denom(b, h1, a_sb, pdf, False, True)
                p_o = po_pool.tile([128, 512], f32, tag="po")
                emit_av(b, h0, a_sb, p_o)
                emit_av(b, h1, a_sb, p_o)
                with nc.allow_low_precision(reason="bf16 softmax recip"):
                    nc.vector.reciprocal(
                        out=r_hh[0:1, b, :, :], in_=pdf[0:1, 0 : 2 * N]
                    )
                # evacuate PSUM unnormalized; the batched normalize happens
                # after the per-head broadcast bounce below
                nc.vector.tensor_copy(
                    out=o2_all[:, hh, b * N : (b + 1) * N],
                    in_=p_o[:, 0:N],
                )

            def bounce(half, r_hh=r_hh, rb_hh=rb_hh, hh=hh):
                """Partition-broadcast the reciprocals of 4 windows with a
                DRAM bounce, then normalize o2 in place on the (otherwise
                idle, SBUF-only) GpSimd engine."""
                HB = BPC // 2
                bs = slice(half * HB, (half + 1) * HB)
                dd = rd_pool.tile([2, HB, N], bf16, tag="rd")
                nc.sync.dma_start(out=dd[0:1, :, :], in_=r_hh[0:1, bs, 0, :])
                nc.sync.dma_start(out=dd[1:2, :, :], in_=r_hh[0:1, bs, 1, :])
                nc.sync.dma_start(
                    out=rb_hh[0:64, bs, :],
                    in_=dd[0:1, :, :].broadcast_to([64, HB, N]),
                )
                nc.sync.dma_start(
                    out=rb_hh[64:128, bs, :],
                    in_=dd[1:2, :, :].broadcast_to([64, HB, N]),
                )
                o2v = o2_all[:, hh, :].rearrange("p (b q) -> p b q", b=BPC)
                nc.gpsimd.tensor_tensor(
                    out=o2v[:, bs, :],
                    in0=o2v[:, bs, :],
                    in1=rb_hh[:, bs, :],
                    op=ALU.mult,
                )

            pend = None
            for b in range(BPC):
                # both parities of the head-pair share one 2-bank psum tile
                # so a single exp instruction covers all four QK chunks;
                # emission runs one window ahead of the post-exp work so the
                # PE queue never stalls behind ScalarE.
                p_a = pa_pool.tile([128, 2, 2, 256], f32, tag="pa")
                a_sb = attn_pool.tile([128, 2, 2, N], bf16, tag="a")
                emit_qk(b, h0, p_a)
                emit_qk(b, h1, p_a)
                nc.scalar.activation(
                    out=a_sb[:], in_=p_a[:, :, :, 0:N], func=AF.Exp, scale=1.0
                )
                if pend is not None:
                    flush(*pend)
                pend = (b, a_sb)
                if b == BPC // 2:
                    bounce(0)
            flush(*pend)
            bounce(1)

        pd_cm.__exit__(None, None, None)
        po_cm.__exit__(None, None, None)
        pa_cm.__exit__(None, None, None)

        # ---- phase 4: projection, feature-major output ----
        # lhsT = proj weights (stationary), rhs = o2 token stream; the bias
        # is per-partition here so ScalarE adds it while evacuating PSUM.
        pp_cm = tc.tile_pool(name="pp", bufs=3, space="PSUM")
        pp_pool = pp_cm.__enter__()
        TCH = [(0, 512), (512, 512), (1024, 512), (1536, 32)]
        for fc in range(DC):
            o_sb = osb_pool.tile([128, T], f32, tag="osb")
            for t0, tn in TCH:
                p_p = pp_pool.tile([128, 512], f32, tag="pp")
                for cc in range(DC):
                    nc.tensor.matmul(
                        p_p[:, 0:tn],
                        lhsT=pw_sb[:, cc, fc * 128 : (fc + 1) * 128],
                        rhs=o2_all[:, cc, t0 : t0 + tn],
                        start=(cc == 0),
                        stop=(cc == DC - 1),
                    )
                nc.scalar.activation(
                    out=o_sb[:, t0 : t0 + tn],
                    in_=p_p[:, 0:tn],
                    func=AF.Identity,
                    bias=bp_sb[:, fc : fc + 1],
                    scale=1.0,
                )
                if t0 + tn in (1024, T):
                    # split the writeback so the last-chunk drain is short
                    w0 = 0 if t0 + tn == 1024 else 1024
                    nc.sync.dma_start(
                        out=out[fc * 128 : (fc + 1) * 128, w0 : t0 + tn],
                        in_=o_sb[:, w0 : t0 + tn],
                    )
        pp_cm.__exit__(None, None, None)

    nc.finalize()
    return nc


def _host_prep(inputs):
    bf16 = ml_dtypes.bfloat16
    x = np.asarray(inputs["x"], np.float32)
    qkv_w = np.asarray(inputs["qkv_w"], np.float32)
    qkv_b = np.asarray(inputs["qkv_b"], np.float32)
    proj_w = np.asarray(inputs["proj_w"], np.float32)
    proj_b = np.asarray(inputs["proj_b"], np.float32)
    la_q = np.asarray(inputs["la_q"], np.float32)
    lb_q = np.asarray(inputs["lb_q"], np.float32)
    la_v = np.asarray(inputs["la_v"], np.float32)
    lb_v = np.asarray(inputs["lb_v"], np.float32)
    rel_pos_h = np.asarray(inputs["rel_pos_h"], np.float32)
    rel_pos_w = np.asarray(inputs["rel_pos_w"], np.float32)

    Wq = qkv_w[:DIM] + lb_q @ la_q
    Wk = qkv_w[DIM : 2 * DIM]
    Wv = qkv_w[2 * DIM :] + lb_v @ la_v

    wqk_host = np.ascontiguousarray(
        np.concatenate([SCALE * Wq, Wk], 0).T.astype(bf16)
    )
    wv_host = np.ascontiguousarray(Wv.T.astype(bf16))
    pw_host = np.ascontiguousarray(proj_w.T.astype(bf16))
    bq_host = (SCALE * qkv_b[:DIM]).astype(np.float32)
    bv_host = np.ascontiguousarray(qkv_b[2 * DIM :].astype(bf16))
    bp_host = proj_b.astype(np.float32)

    idx = np.arange(H)[:, None] - np.arange(H)[None, :] + (H - 1)
    Rh = rel_pos_h[idx]  # [qh, kh_j, hd]
    Rw = rel_pos_w[idx]  # [qw, kw_j, hd]
    relh_host = np.ascontiguousarray(
        (Rh / SCALE).transpose(2, 0, 1).reshape(HD, N).astype(bf16)
    )
    relw_host = np.ascontiguousarray(
        (Rw / SCALE).transpose(2, 0, 1).reshape(HD, N).astype(bf16)
    )

    kt = np.arange(N)
    oh_kh = (kt[None, :] // W == np.arange(H)[:, None]).astype(bf16)  # [14, 196]
    oh_kw = (kt[None, :] % W == np.arange(W)[:, None]).astype(bf16)
    z18 = np.zeros((18, N), bf16)
    oh_e_1 = np.concatenate([oh_kh, z18, oh_kw], 0)  # [46, 196]
    oh_o_1 = np.concatenate([oh_kw, z18, oh_kh, z18], 0)  # [64, 196]
    # replicate across the 48 (window, head-pair) slots -> contiguous DMA
    oh_e_host = np.ascontiguousarray(
        np.broadcast_to(oh_e_1[:, None, :], (46, NPR, N)).reshape(46, NPR * N)
    )
    oh_o_host = np.ascontiguousarray(
        np.broadcast_to(oh_o_1[:, None, :], (64, NPR, N)).reshape(64, NPR * N)
    )

    shared = {
        "wqk": wqk_host,
        "wv": wv_host,
        "pw": pw_host,
        "bq": bq_host,
        "bv": bv_host,
        "bp": bp_host,
        "relh": relh_host,
        "relw": relw_host,
        "oh_e": oh_e_host,
        "oh_o": oh_o_host,
    }

    x_flat = x.reshape(B_TOTAL, N, DIM)
    in_maps = []
    for c in range(NCORES):
        xc = x_flat[c * BPC : (c + 1) * BPC].reshape(BPC * N, DIM)
        xT_c = np.ascontiguousarray(xc.T.astype(bf16))
        m = dict(shared)
        m["xT"] = xT_c
        in_maps.append(m)
    return in_maps


def kernel(**inputs):
    from concourse import bass_utils

    if "nc" not in _NC_CACHE:
        _NC_CACHE["nc"] = build_module()
    nc = _NC_CACHE["nc"]
    in_maps = _host_prep(inputs)
    res = bass_utils.run_bass_kernel_spmd(
        nc, in_maps, core_ids=list(range(NCORES))
    )
    outs = [
        np.ascontiguousarray(r["out"].T).reshape(BPC, H, W, DIM)
        for r in res.results
    ]
    return np.concatenate(outs, 0)
